# revision 21
# baseline (speedup 1.0000x reference)
"""Trainium2 Bass kernel for CDMamba ModifiedSRCMLayer (self-contained).

Sharding: 8 cores; core k handles batch k//2 and mamba group-pair k%2
(groups {0,1} or {2,3}). Group outputs are exchanged with a paired
AllGather; the post-stage (gate blend + output projection) is computed
redundantly on both cores of a pair and the host reads even cores.

Selective scan runs on the DVE via tensor_tensor_scan over tiles of
[128 partitions = 2 s-values x 64 d, 512 timesteps]; exp(dt*A) on the
scalar engine with per-partition scale; B/C broadcasts, the s-reduction,
convolutions, and projections on the tensor engine. The backward
direction uses negative-step APs (free reversal).
"""
import sys
import numpy as np

for _p in ("/opt/trn_rl_repo",):
    if _p not in sys.path:
        sys.path.append(_p)

import concourse.bass as bass
import concourse.mybir as mybir
from concourse.bacc import Bacc
from concourse.tile import TileContext
from concourse.bass_types import AP as _AP

# Model dims (hardcoded per the problem spec)
B, C, H, W = 4, 128, 64, 64
L = H * W                      # 4096
G, DM = 4, 32
DI, DS, DC = 64, 16, 4
DTR = 2
OUT = 128
EPS = 1e-5

NCORE = 8
LC = 512                       # time chunk
NCH = L // LC                  # 8
NJ = DS // 2                   # 8 s-tiles (2 s-values per tile)
F32 = mybir.dt.float32
BF = mybir.dt.bfloat16
AF = mybir.ActivationFunctionType
ALU = mybir.AluOpType


def _build_nc():
    nc = Bacc(num_devices=NCORE)

    def inp(name, shape, dt=F32):
        return nc.dram_tensor(name, list(shape), dt, kind="ExternalInput")

    # per-core data
    xpad = inp("xpad", (C, 66 * 66))
    pe_b = inp("pe_b", (C, L))
    # weights (already laid out per core-set on the host)
    w9 = inp("w9", (C, 9 * 128))
    mred1 = inp("mred1", (128, 1))
    onesr = inp("onesr", (1, 128))
    ln_g = inp("ln_g", (128, 1))
    ln_b = inp("ln_b", (128, 1))
    gateWT = inp("gateWT", (128, 128))
    gateb = inp("gateb", (128, 1))
    winTu = inp("winTu", (2, C, DI))    # group-select baked in (zero rows)
    winTz = inp("winTz", (2, C, DI))
    conv4T = inp("conv4T", (2, 2, DC, DI, 128), BF)
    convb = inp("convb", (2, 2, 128, 1))
    dtWT = inp("dtWT", (2, 2, DI, 128), BF)
    dtb = inp("dtb", (2, 2, 128, 1))
    xprojBCT = inp("xprojBCT", (2, 2, DI, 2 * DS), BF)
    A_sc = inp("A_sc", (2, 2, 128, NJ))
    mredM = inp("mredM", (128, DI), BF)
    dsk = inp("dsk", (2, 2, 128, 1))
    woutT = inp("woutT", (128, 2 * DM), BF)
    projT = inp("projT", (128, 128))
    projb = inp("projb", (128, 1))

    xm_loc = nc.dram_tensor("xm_loc", [2 * DM, L], F32)
    xm_all = nc.dram_tensor("xm_all", [C, L], F32)
    outp = nc.dram_tensor("outp", [OUT, L], F32, kind="ExternalOutput")

    with TileContext(nc) as tc:
        with (
            tc.tile_pool(name="const", bufs=1) as cp,
            tc.tile_pool(name="big", bufs=1) as bp,
            tc.tile_pool(name="hpool", bufs=2) as hp,
            tc.tile_pool(name="psP", bufs=1, space="PSUM") as psP,
        ):
            # ---- load constants to SBUF ----
            def c_load(ap_dram, shape, nm):
                t = cp.tile(list(shape), F32, name=nm, tag=nm)
                nc.sync.dma_start(t[:], ap_dram)
                return t

            w9_sb = c_load(w9[:], (C, 9 * 128), "w9sb")
            mred1_sb = c_load(mred1[:], (128, 1), "mred1sb")
            onesr_sb = c_load(onesr[:], (1, 128), "onesrsb")
            lng_sb = c_load(ln_g[:], (128, 1), "lngsb")
            lnb_sb = c_load(ln_b[:], (128, 1), "lnbsb")
            gateWT_sb = c_load(gateWT[:], (128, 128), "gateWTsb")
            gateb_sb = c_load(gateb[:], (128, 1), "gatebsb")
            mredM_sb = cp.tile([128, DI], BF, name="mredMsb", tag="mredMsb")
            nc.sync.dma_start(mredM_sb[:], mredM[:])
            woutT_sb = cp.tile([128, 2 * DM], BF, name="woutTsb", tag="woutTsb")
            nc.sync.dma_start(woutT_sb[:], woutT[:])
            projT_sb = c_load(projT[:], (128, 128), "projTsb")
            projb_sb = c_load(projb[:], (128, 1), "projbsb")

            winTu_sb = cp.tile([C, 2 * DI], F32)
            winTz_sb = cp.tile([C, 2 * DI], F32)
            conv4T_sb = cp.tile([128, 16 * 128], BF)
            dtWT_sb = cp.tile([DI, 4 * 128], BF)
            xprojBCT_sb = cp.tile([DI, 4 * 2 * DS], BF)
            asc_sb = cp.tile([128, 4 * NJ], F32)
            convb_sb = cp.tile([128, 4], F32)
            dtb_sb = cp.tile([128, 4], F32)
            dsk_sb = cp.tile([128, 4], F32)
            eps_sb = cp.tile([1, 1], F32)
            nc.vector.memset(eps_sb[:], EPS)
            for gl in range(2):
                nc.sync.dma_start(winTu_sb[:, gl * DI:(gl + 1) * DI], winTu[gl])
                nc.sync.dma_start(winTz_sb[:, gl * DI:(gl + 1) * DI], winTz[gl])
                for dr in range(2):
                    i4 = gl * 2 + dr
                    for k in range(DC):
                        for hh in range(2):
                            nc.sync.dma_start(
                                conv4T_sb[hh * 64:(hh + 1) * 64,
                                          (i4 * 4 + k) * 128:(i4 * 4 + k + 1) * 128],
                                conv4T[gl, dr, k])
                    nc.sync.dma_start(dtWT_sb[:, i4 * 128:(i4 + 1) * 128], dtWT[gl, dr])
                    nc.sync.dma_start(
                        xprojBCT_sb[:, i4 * 2 * DS:(i4 + 1) * 2 * DS], xprojBCT[gl, dr])
                    nc.sync.dma_start(asc_sb[:, i4 * NJ:(i4 + 1) * NJ], A_sc[gl, dr])
                    nc.sync.dma_start(convb_sb[:, i4:i4 + 1], convb[gl, dr])
                    nc.sync.dma_start(dtb_sb[:, i4:i4 + 1], dtb[gl, dr])
                    nc.sync.dma_start(dsk_sb[:, i4:i4 + 1], dsk[gl, dr])

            # ---- big persistent tiles ----
            xs = bp.tile([C, L], F32)       # post pos-embed input, (c, l) layout
            gate = bp.tile([C, L], F32)
            u_pad = bp.tile([C, L + 6], BF)  # rows [g0 u | g1 u]; 3-zero halo
            zs = bp.tile([C, L], BF)       # silu(z), group-packed rows
            yfb = bp.tile([C, L], BF)      # y_fwd + y_bwd, group-packed rows

            nc.vector.memset(u_pad[:, 0:3], 0.0)
            nc.vector.memset(u_pad[:, L + 3:L + 6], 0.0)

            # ---- Phase A: conv-pos-enc + pos-embed + LN (pass 1), then
            # gate + xz (pass 2) — two passes so ACT table sets batch ----
            with tc.tile_pool(name="pA", bufs=2) as pA:
                xpad_sb = pA.tile([C, 66 * 66], F32, bufs=1)
                nc.sync.dma_start(xpad_sb[:], xpad[:])
                xpad3 = xpad_sb[:].rearrange("p (r q) -> p r q", q=66)
                xnc = pA.tile([C, L], F32, bufs=1)
                for c in range(NCH):
                    cs = slice(c * LC, (c + 1) * LC)
                    pa = psP.tile([128, 8, 64], F32, tag="gen", bufs=2)
                    for tap in range(9):
                        dy, dx = tap // 3, tap % 3
                        nc.tensor.matmul(
                            pa[:],
                            w9_sb[:, tap * 128:(tap + 1) * 128],
                            xpad3[:, c * 8 + dy:c * 8 + dy + 8, dx:dx + 64],
                            start=(tap == 0), stop=(tap == 8))
                    paf = pa[:].rearrange("p a b -> p (a b)")
                    pe_t = pA.tile([128, LC], F32, tag="pe")
                    nc.sync.dma_start(pe_t[:], pe_b[:, cs])
                    nc.vector.tensor_tensor(xs[:, cs], paf, pe_t[:], op=ALU.add)

                    mu = psP.tile([1, LC], F32, tag="gen", bufs=2)
                    nc.tensor.matmul(mu[:], mred1_sb[:], xs[:, cs],
                                     start=True, stop=True)
                    mu_sb = pA.tile([1, LC], F32, tag="musb")
                    nc.scalar.copy(mu_sb[:], mu[:])
                    mub = psP.tile([128, LC], F32, tag="gen", bufs=2)
                    nc.tensor.matmul(mub[:], onesr_sb[:], mu_sb[:],
                                     start=True, stop=True)
                    xc = pA.tile([128, LC], F32, tag="xc")
                    nc.vector.tensor_tensor(xc[:], xs[:, cs], mub[:], op=ALU.subtract)
                    xsq = pA.tile([128, LC], F32, tag="xsq")
                    nc.scalar.square(xsq[:], xc[:])
                    var = psP.tile([1, LC], F32, tag="gen", bufs=2)
                    nc.tensor.matmul(var[:], mred1_sb[:], xsq[:], start=True, stop=True)
                    sd = pA.tile([1, LC], F32, tag="sd")
                    nc.scalar.activation(sd[:], var[:], AF.Sqrt, bias=eps_sb[:, 0:1])
                    rstd = pA.tile([1, LC], F32, tag="rstd")
                    nc.vector.reciprocal(rstd[:], sd[:])
                    rstdb = psP.tile([128, LC], F32, tag="gen", bufs=2)
                    nc.tensor.matmul(rstdb[:], onesr_sb[:], rstd[:],
                                     start=True, stop=True)
                    xng = pA.tile([128, LC], F32, tag="xng")
                    nc.vector.tensor_tensor(xng[:], xc[:], rstdb[:], op=ALU.mult)
                    nc.scalar.activation(xnc[:, cs], xng[:], AF.Identity,
                                         bias=lnb_sb[:, 0:1], scale=lng_sb[:, 0:1])

                for c in range(NCH):
                    cs = slice(c * LC, (c + 1) * LC)
                    gps = psP.tile([128, LC], F32, tag="gen", bufs=2)
                    nc.tensor.matmul(gps[:], gateWT_sb[:], xnc[:, cs],
                                     start=True, stop=True)
                    nc.scalar.activation(gate[:, cs], gps[:], AF.Sigmoid,
                                         bias=gateb_sb[:, 0:1])
                    for gl in range(2):
                        rows = slice(gl * 64, gl * 64 + 64)
                        xzp = psP.tile([128, LC], F32, tag="gen", bufs=2)
                        nc.tensor.matmul(xzp[rows, :],
                                         winTu_sb[:, gl * DI:(gl + 1) * DI],
                                         xnc[:, cs], start=True, stop=True)
                        nc.scalar.copy(u_pad[rows, 3 + c * LC: 3 + (c + 1) * LC],
                                       xzp[rows, :])
                        xzp2 = psP.tile([128, LC], F32, tag="gen", bufs=2)
                        nc.tensor.matmul(xzp2[rows, :],
                                         winTz_sb[:, gl * DI:(gl + 1) * DI],
                                         xnc[:, cs], start=True, stop=True)
                        sgz = pA.tile([128, LC], BF, tag="sgz")
                        nc.scalar.activation(sgz[rows, :], xzp2[rows, :], AF.Sigmoid)
                        nc.vector.scalar_tensor_tensor(
                            zs[rows, cs], xzp2[rows, :], 0.0, sgz[rows, :],
                            op0=ALU.add, op1=ALU.mult)

            # ---- Phase B: per (group, direction, L-half) front-end + scan ----
            LH = L // 2
            NCC = LH // LC  # 4 front-end chunks per half
            with tc.tile_pool(name="pB", bufs=2) as wp:
                for gl in range(2):
                    rows = slice(gl * 64, gl * 64 + 64)
                    for dr in range(2):
                        i4 = gl * 2 + dr
                        h_prev = [None] * NJ
                        horder = (0, 1) if dr == 0 else (1, 0)
                        for hf in horder:
                            uc_h = wp.tile([128, LH], BF, tag="uc_h", bufs=2)
                            sgd_h = wp.tile([128, LH], BF, tag="sgd_h", bufs=2)
                            bc_h = wp.tile([DS, 2 * LH], BF, tag="bc_h", bufs=2)
                            # front-end (natural order); sigmoid table set
                            for cc in range(NCC):
                                c = hf * NCC + cc
                                ccs = slice(cc * LC, (cc + 1) * LC)
                                ucp = psP.tile([128, LC], F32, tag="gen", bufs=2)
                                for k in range(DC):
                                    off = (c * LC + k) if dr == 0 else (3 + c * LC + k)
                                    nc.tensor.matmul(
                                        ucp[:],
                                        conv4T_sb[rows,
                                                  (i4 * 4 + k) * 128:
                                                  (i4 * 4 + k + 1) * 128],
                                        u_pad[rows, off:off + LC],
                                        start=(k == 0), stop=(k == DC - 1))
                                sgu = wp.tile([128, LC], BF, tag="sgu")
                                nc.scalar.activation(sgu[:], ucp[:], AF.Sigmoid,
                                                     bias=convb_sb[:, i4:i4 + 1])
                                nc.vector.scalar_tensor_tensor(
                                    uc_h[:, ccs], ucp[:], convb_sb[:, i4:i4 + 1],
                                    sgu[:], op0=ALU.add, op1=ALU.mult)
                                dtp = psP.tile([128, LC], F32, tag="gen", bufs=2)
                                nc.tensor.matmul(dtp[:],
                                                 dtWT_sb[:, i4 * 128:(i4 + 1) * 128],
                                                 uc_h[0:DI, ccs],
                                                 start=True, stop=True)
                                nc.scalar.activation(sgd_h[:, ccs], dtp[:], AF.Sigmoid,
                                                     bias=dtb_sb[:, i4:i4 + 1],
                                                     scale=-1.0)
                                bcp = psP.tile([2 * DS, 2 * LC], F32, tag="bcp2",
                                               bufs=1)
                                nc.tensor.matmul(
                                    bcp[0:DS, 0:LC],
                                    xprojBCT_sb[:, i4 * 2 * DS:i4 * 2 * DS + DS],
                                    uc_h[0:DI, ccs], start=True, stop=True)
                                nc.tensor.matmul(
                                    bcp[0:DS, LC:2 * LC],
                                    xprojBCT_sb[:, i4 * 2 * DS + DS:(i4 + 1) * 2 * DS],
                                    uc_h[0:DI, ccs], start=True, stop=True)
                                bco = _AP(tensor=bc_h[:].tensor, offset=bc_h[:].offset
                                          + cc * LC,
                                          ap=[[2 * LH, DS], [LH, 2], [1, LC]])
                                nc.scalar.copy(bco, bcp[0:DS, :])
                            # lnexp table set from here on
                            nc.scalar.activation(sgd_h[:], sgd_h[:], AF.Ln)
                            dt_h = sgd_h
                            dtuc = wp.tile([128, LH], BF, tag="dtuc", bufs=2)
                            nc.vector.tensor_tensor(dtuc[:], dt_h[:], uc_h[:],
                                                    op=ALU.mult)
                            ys = [psP.tile([128, LC], F32, tag=f"ys{q}", bufs=1,
                                           name=f"ys{q}")
                                  for q in range(NCC)]
                            for j in range(NJ):
                                dA = wp.tile([128, LH], BF, tag="dA")
                                nc.scalar.activation(
                                    dA[:], dt_h[:], AF.Exp,
                                    scale=asc_sb[:, i4 * NJ + j:i4 * NJ + j + 1])
                                bbs = wp.tile([128, 2 * LH], BF, tag="bbs")
                                bsrc = bc_h[2 * j:2 * j + 2, :]
                                brep = _AP(tensor=bsrc.tensor, offset=bsrc.offset,
                                           ap=[[bsrc.ap[0][0], 2], [0, 64],
                                               [1, 2 * LH]])
                                dma_eng = (nc.sync, nc.gpsimd)[j % 2]
                                dma_eng.dma_start(bbs[:], brep)
                                dBu = wp.tile([128, LH], BF, tag="dBu")
                                nc.vector.tensor_tensor(dBu[:], dtuc[:],
                                                        bbs[:, 0:LH], op=ALU.mult)
                                h = hp.tile([128, LH], BF, tag="h")
                                first = (hf == horder[0])
                                hc = hp.tile([128, 1], BF, tag=f"hc{j}",
                                             name=f"hc{j}")
                                if dr == 0:
                                    init = 0.0 if first else h_prev[j][:, 0:1]
                                    nc.vector.tensor_tensor_scan(
                                        h[:], dA[:], dBu[:], init,
                                        op0=ALU.mult, op1=ALU.add)
                                    nc.scalar.copy(hc[:], h[:, LH - 1:LH])
                                else:
                                    init = 0.0 if first else h_prev[j][:, 0:1]
                                    nc.vector.tensor_tensor_scan(
                                        h[:, ::-1], dA[:, ::-1], dBu[:, ::-1], init,
                                        op0=ALU.mult, op1=ALU.add)
                                    nc.scalar.copy(hc[:], h[:, 0:1])
                                h_prev[j] = hc
                                prod = wp.tile([128, LH], BF, tag="prod")
                                nc.vector.tensor_tensor(prod[:], h[:],
                                                        bbs[:, LH:2 * LH],
                                                        op=ALU.mult)
                                for q in range(NCC):
                                    nc.tensor.matmul(
                                        ys[q][rows, :], mredM_sb[:, 0:DI],
                                        prod[:, q * LC:(q + 1) * LC],
                                        start=(j == 0), stop=(j == NJ - 1))
                            for q in range(NCC):
                                c = hf * NCC + q
                                cs = slice(c * LC, (c + 1) * LC)
                                ccs = slice(q * LC, (q + 1) * LC)
                                y1 = wp.tile([128, LC], BF, tag="y1")
                                nc.vector.scalar_tensor_tensor(
                                    y1[rows, :], uc_h[rows, ccs],
                                    dsk_sb[rows, i4:i4 + 1],
                                    ys[q][rows, :], op0=ALU.mult, op1=ALU.subtract)
                                if dr == 0:
                                    nc.vector.tensor_tensor(yfb[rows, cs],
                                                            y1[rows, :],
                                                            zs[rows, cs],
                                                            op=ALU.mult)
                                else:
                                    y2 = wp.tile([128, LC], BF, tag="y2")
                                    nc.vector.tensor_tensor(y2[rows, :], y1[rows, :],
                                                            zs[rows, cs],
                                                            op=ALU.mult)
                                    nc.vector.tensor_tensor(yfb[rows, cs],
                                                            yfb[rows, cs],
                                                            y2[rows, :], op=ALU.add)

            # ---- Phase C: Wout, exchange, blend, proj ----
            with tc.tile_pool(name="pC", bufs=2) as wpc:
                for c in range(NCH):
                    cs = slice(c * LC, (c + 1) * LC)
                    ymp = psP.tile([2 * DM, LC], F32, tag="gen", bufs=2)
                    nc.tensor.matmul(ymp[:], woutT_sb[:], yfb[:, cs],
                                     start=True, stop=True)
                    ym_sb = wpc.tile([2 * DM, LC], F32, tag="ymsb")
                    nc.scalar.copy(ym_sb[:], ymp[:])
                    nc.sync.dma_start(xm_loc[:, cs], ym_sb[:])
                nc.gpsimd.collective_compute(
                    "AllGather", ALU.bypass,
                    replica_groups=[[0, 1], [2, 3], [4, 5], [6, 7]],
                    ins=[xm_loc[:]], outs=[xm_all[:]])
                for c in range(NCH):
                    cs = slice(c * LC, (c + 1) * LC)
                    xm_t = wpc.tile([C, LC], F32, tag="xmt")
                    nc.sync.dma_start(xm_t[:], xm_all[:, cs])
                    ta = wpc.tile([128, LC], F32, tag="ta")
                    nc.vector.tensor_tensor(ta[:], xm_t[:], xs[:, cs],
                                            op=ALU.subtract)
                    tb2 = wpc.tile([128, LC], F32, tag="tb")
                    nc.vector.tensor_tensor(tb2[:], gate[:, cs], ta[:], op=ALU.mult)
                    tc2 = wpc.tile([128, LC], F32, tag="tc")
                    nc.vector.tensor_tensor(tc2[:], xs[:, cs], tb2[:], op=ALU.add)
                    op_ = psP.tile([128, LC], F32, tag="gen", bufs=2)
                    nc.tensor.matmul(op_[:], projT_sb[:], tc2[:], start=True, stop=True)
                    osb = wpc.tile([128, LC], F32, tag="osb")
                    nc.scalar.activation(osb[:], op_[:], AF.Identity,
                                         bias=projb_sb[:, 0:1])
                    nc.sync.dma_start(outp[:, cs], osb[:])
    nc.finalize()
    return nc


def _bf(a):
    import concourse.mybir as _mb
    return np.asarray(a).astype(_mb.dt.np(_mb.dt.bfloat16))


def _prep_inputs(inputs):
    """Build the 8 per-core in_maps from full inputs."""
    ii = {k: np.asarray(v, dtype=np.float32) for k, v in inputs.items()}
    x = ii["x"]

    maps_w = []  # weight dicts per group-set gs=0,1
    for gs in range(2):
        w = {}
        w9 = np.zeros((C, 9 * 128), np.float32)
        for tap in range(9):
            dy, dx = tap // 3, tap % 3
            blk = np.zeros((C, 128), np.float32)
            np.fill_diagonal(blk, ii["pos_conv_w"][:, 0, dy, dx])
            if tap == 4:
                blk[np.arange(C), np.arange(C)] += 1.0
            w9[:, tap * 128:(tap + 1) * 128] = blk
        w["w9"] = w9
        w["pe_b"] = np.ascontiguousarray(ii["pos_embed"][0].T) \
            + ii["pos_conv_b"][:, None]
        w["mred1"] = np.full((128, 1), 1.0 / 128, np.float32)
        w["onesr"] = np.ones((1, 128), np.float32)
        w["ln_g"] = np.ascontiguousarray(ii["ln_g"][:, None])
        w["ln_b"] = np.ascontiguousarray(ii["ln_b"][:, None])
        w["gateWT"] = np.ascontiguousarray(ii["gate_W"].T)
        w["gateb"] = np.ascontiguousarray(ii["gate_b"][:, None])
        w["projT"] = np.ascontiguousarray(ii["proj_W"].T)
        w["projb"] = np.ascontiguousarray(ii["proj_b"][:, None])
        w["mredM"] = _bf(np.tile(np.eye(DI, dtype=np.float32), (2, 1)))
        winTu = np.zeros((2, C, DI), np.float32)
        winTz = np.zeros((2, C, DI), np.float32)
        conv4T = np.zeros((2, 2, DC, DI, 128), np.float32)
        convb = np.zeros((2, 2, 128, 1), np.float32)
        dtWT = np.zeros((2, 2, DI, 128), np.float32)
        dtb = np.zeros((2, 2, 128, 1), np.float32)
        xprojBCT = np.zeros((2, 2, DI, 2 * DS), np.float32)
        A_sc = np.zeros((2, 2, 128, NJ), np.float32)
        dsk = np.zeros((2, 2, 128, 1), np.float32)
        woutT = np.zeros((128, 2 * DM), np.float32)
        for gl in range(2):
            gg = gs * 2 + gl
            gsl = slice(gg * DM, (gg + 1) * DM)
            winTu[gl, gsl, :] = ii["m_Win"][gg, 0:DI, :].T
            winTz[gl, gsl, :] = ii["m_Win"][gg, DI:2 * DI, :].T
            woutT[gl * 64:(gl + 1) * 64, gl * DM:(gl + 1) * DM] = ii["m_Wout"][gg].T
            for dr in range(2):
                for k in range(DC):
                    wk = ii["conv_w"][gg, dr, :, k if dr == 0 else DC - 1 - k]
                    blk = np.zeros((DI, 128), np.float32)
                    blk[np.arange(DI), np.arange(DI)] = wk
                    blk[np.arange(DI), 64 + np.arange(DI)] = wk
                    conv4T[gl, dr, k] = blk
                convb[gl, dr, :, 0] = np.tile(ii["conv_b"][gg, dr], 2)
                M2 = ii["dt_W"][gg, dr] @ ii["xproj_W"][gg, dr][0:DTR, :]  # (DI, DI)
                dtWT[gl, dr] = np.concatenate([M2.T, M2.T], axis=1)  # [DI, 128]
                dtb[gl, dr, :, 0] = -np.tile(ii["dt_b"][gg, dr], 2)
                xprojBCT[gl, dr] = ii["xproj_W"][gg, dr][DTR:DTR + 2 * DS, :].T
                A = np.exp(ii["A_log"][gg, dr])  # (DI, DS); dt is negated, so +exp
                p = np.arange(128)
                for j in range(NJ):
                    A_sc[gl, dr, :, j] = A[p % 64, 2 * j + p // 64]
                dsk[gl, dr, :, 0] = np.tile(ii["Dskip"][gg, dr], 2)
        w.update(winTu=winTu, winTz=winTz, conv4T=_bf(conv4T), convb=convb,
                 dtWT=_bf(dtWT), dtb=dtb, xprojBCT=_bf(xprojBCT), A_sc=A_sc,
                 dsk=dsk, woutT=_bf(woutT))
        maps_w.append(w)

    in_maps = []
    for k in range(NCORE):
        b, gs = k // 2, k % 2
        m = dict(maps_w[gs])
        xp = np.zeros((C, 66, 66), np.float32)
        xp[:, 1:65, 1:65] = x[b]
        m["xpad"] = np.ascontiguousarray(xp.reshape(C, 66 * 66))
        in_maps.append(m)
    return in_maps


_CACHE = {}


def kernel(**inputs):
    from concourse.bass_utils import run_bass_kernel_spmd
    if "nc" not in _CACHE:
        _CACHE["nc"] = _build_nc()
    nc = _CACHE["nc"]
    in_maps = _prep_inputs(inputs)
    res = run_bass_kernel_spmd(nc, in_maps, list(range(NCORE))).results
    out = np.stack([np.asarray(res[2 * b]["outp"]).reshape(OUT, H, W)
                    for b in range(B)])
    return out.astype(np.float32)


# revision 22
# speedup vs baseline: 2.0888x; 2.0888x over previous
"""Trainium2 Bass kernel for CDMamba ModifiedSRCMLayer (self-contained).

Sharding: 8 cores; core k handles batch k//2 and mamba group-pair k%2
(groups {0,1} or {2,3}). Group outputs are exchanged with a paired
AllGather; the post-stage (gate blend + output projection) is computed
redundantly on both cores of a pair and the host reads even cores.

Selective scan runs on the DVE via tensor_tensor_scan over tiles of
[128 partitions = 2 s-values x 64 d, 512 timesteps]; exp(dt*A) on the
scalar engine with per-partition scale; B/C broadcasts, the s-reduction,
convolutions, and projections on the tensor engine. The backward
direction uses negative-step APs (free reversal).
"""
import sys
import numpy as np

for _p in ("/opt/trn_rl_repo",):
    if _p not in sys.path:
        sys.path.append(_p)

import concourse.bass as bass
import concourse.mybir as mybir
from concourse.bacc import Bacc
from concourse.tile import TileContext
from concourse.bass_types import AP as _AP

# Model dims (hardcoded per the problem spec)
B, C, H, W = 4, 128, 64, 64
L = H * W                      # 4096
G, DM = 4, 32
DI, DS, DC = 64, 16, 4
DTR = 2
OUT = 128
EPS = 1e-5

NCORE = 8
LC = 512                       # time chunk
NCH = L // LC                  # 8
NJ = DS // 2                   # 8 s-tiles (2 s-values per tile)
F32 = mybir.dt.float32
BF = mybir.dt.bfloat16
AF = mybir.ActivationFunctionType
ALU = mybir.AluOpType


def _build_nc():
    nc = Bacc(num_devices=NCORE)

    def inp(name, shape, dt=F32):
        return nc.dram_tensor(name, list(shape), dt, kind="ExternalInput")

    # per-core data
    xpad = inp("xpad", (C, 66 * 66))
    pe_b = inp("pe_b", (C, L))
    # weights (already laid out per core-set on the host)
    w9 = inp("w9", (C, 9 * 128))
    mred1 = inp("mred1", (128, 1))
    onesr = inp("onesr", (1, 128))
    ln_g = inp("ln_g", (128, 1))
    ln_b = inp("ln_b", (128, 1))
    gateWT = inp("gateWT", (128, 128))
    gateb = inp("gateb", (128, 1))
    winTu = inp("winTu", (2, C, DI))    # group-select baked in (zero rows)
    winTz = inp("winTz", (2, C, DI))
    conv4T = inp("conv4T", (2, 2, DC, DI, 128), BF)
    convb = inp("convb", (2, 2, 128, 1))
    dtWT = inp("dtWT", (2, 2, DI, 128), BF)
    dtb = inp("dtb", (2, 2, 128, 1))
    xprojBCT = inp("xprojBCT", (2, 2, DI, 2 * DS), BF)
    A_sc = inp("A_sc", (2, 2, 128, NJ))
    mredM = inp("mredM", (128, DI), BF)
    dsk = inp("dsk", (2, 2, 128, 1))
    woutT = inp("woutT", (128, 2 * DM), BF)
    projT = inp("projT", (128, 128))
    projb = inp("projb", (128, 1))

    xm_loc = nc.dram_tensor("xm_loc", [2 * DM, L], F32)
    bc_dram = nc.dram_tensor("bc_dram", [4, DS, L], BF)
    xm_all = nc.dram_tensor("xm_all", [C, L], F32)
    outp = nc.dram_tensor("outp", [OUT, L], F32, kind="ExternalOutput")

    with TileContext(nc) as tc:
        with (
            tc.tile_pool(name="const", bufs=1) as cp,
            tc.tile_pool(name="big", bufs=1) as bp,
            tc.tile_pool(name="hpool", bufs=2) as hp,
            tc.tile_pool(name="psP", bufs=1, space="PSUM") as psP,
        ):
            # ---- load constants to SBUF ----
            def c_load(ap_dram, shape, nm):
                t = cp.tile(list(shape), F32, name=nm, tag=nm)
                nc.sync.dma_start(t[:], ap_dram)
                return t

            w9_sb = c_load(w9[:], (C, 9 * 128), "w9sb")
            mred1_sb = c_load(mred1[:], (128, 1), "mred1sb")
            onesr_sb = c_load(onesr[:], (1, 128), "onesrsb")
            lng_sb = c_load(ln_g[:], (128, 1), "lngsb")
            lnb_sb = c_load(ln_b[:], (128, 1), "lnbsb")
            gateWT_sb = c_load(gateWT[:], (128, 128), "gateWTsb")
            gateb_sb = c_load(gateb[:], (128, 1), "gatebsb")
            mredM_sb = cp.tile([128, DI], BF, name="mredMsb", tag="mredMsb")
            nc.sync.dma_start(mredM_sb[:], mredM[:])
            woutT_sb = cp.tile([128, 2 * DM], BF, name="woutTsb", tag="woutTsb")
            nc.sync.dma_start(woutT_sb[:], woutT[:])
            projT_sb = c_load(projT[:], (128, 128), "projTsb")
            projb_sb = c_load(projb[:], (128, 1), "projbsb")

            winTu_sb = cp.tile([C, 2 * DI], F32)
            winTz_sb = cp.tile([C, 2 * DI], F32)
            conv4T_sb = cp.tile([128, 16 * 128], BF)
            dtWT_sb = cp.tile([DI, 4 * 128], BF)
            xprojBCT_sb = cp.tile([DI, 4 * 2 * DS], BF)
            asc_sb = cp.tile([128, 4 * NJ], F32)
            convb_sb = cp.tile([128, 4], F32)
            dtb_sb = cp.tile([128, 4], F32)
            dsk_sb = cp.tile([128, 4], F32)
            eps_sb = cp.tile([1, 1], F32)
            nc.vector.memset(eps_sb[:], EPS)
            for gl in range(2):
                nc.sync.dma_start(winTu_sb[:, gl * DI:(gl + 1) * DI], winTu[gl])
                nc.sync.dma_start(winTz_sb[:, gl * DI:(gl + 1) * DI], winTz[gl])
                for dr in range(2):
                    i4 = gl * 2 + dr
                    for k in range(DC):
                        for hh in range(2):
                            nc.sync.dma_start(
                                conv4T_sb[hh * 64:(hh + 1) * 64,
                                          (i4 * 4 + k) * 128:(i4 * 4 + k + 1) * 128],
                                conv4T[gl, dr, k])
                    nc.sync.dma_start(dtWT_sb[:, i4 * 128:(i4 + 1) * 128], dtWT[gl, dr])
                    nc.sync.dma_start(
                        xprojBCT_sb[:, i4 * 2 * DS:(i4 + 1) * 2 * DS], xprojBCT[gl, dr])
                    nc.sync.dma_start(asc_sb[:, i4 * NJ:(i4 + 1) * NJ], A_sc[gl, dr])
                    nc.sync.dma_start(convb_sb[:, i4:i4 + 1], convb[gl, dr])
                    nc.sync.dma_start(dtb_sb[:, i4:i4 + 1], dtb[gl, dr])
                    nc.sync.dma_start(dsk_sb[:, i4:i4 + 1], dsk[gl, dr])

            # ---- big persistent tiles ----
            xs = bp.tile([C, L], F32)       # post pos-embed input, (c, l) layout
            gate = bp.tile([C, L], F32)
            u_pad = bp.tile([C, L + 6], BF)  # rows [g0 u | g1 u]; 3-zero halo
            zs = bp.tile([C, L], BF)       # silu(z), group-packed rows
            yfb = bp.tile([C, L], BF)      # y_fwd + y_bwd, group-packed rows

            nc.vector.memset(u_pad[:, 0:3], 0.0)
            nc.vector.memset(u_pad[:, L + 3:L + 6], 0.0)

            # ---- Phase A: conv-pos-enc + pos-embed + LN (pass 1), then
            # gate + xz (pass 2) — two passes so ACT table sets batch ----
            with tc.tile_pool(name="pA", bufs=2) as pA:
                xpad_sb = pA.tile([C, 66 * 66], F32, bufs=1)
                nc.sync.dma_start(xpad_sb[:], xpad[:])
                xpad3 = xpad_sb[:].rearrange("p (r q) -> p r q", q=66)
                xnc = pA.tile([C, L], F32, bufs=1)
                for c in range(NCH):
                    cs = slice(c * LC, (c + 1) * LC)
                    pa = psP.tile([128, 8, 64], F32, tag="gen", bufs=2)
                    for tap in range(9):
                        dy, dx = tap // 3, tap % 3
                        nc.tensor.matmul(
                            pa[:],
                            w9_sb[:, tap * 128:(tap + 1) * 128],
                            xpad3[:, c * 8 + dy:c * 8 + dy + 8, dx:dx + 64],
                            start=(tap == 0), stop=(tap == 8))
                    paf = pa[:].rearrange("p a b -> p (a b)")
                    pe_t = pA.tile([128, LC], F32, tag="pe")
                    nc.sync.dma_start(pe_t[:], pe_b[:, cs])
                    nc.vector.tensor_tensor(xs[:, cs], paf, pe_t[:], op=ALU.add)

                    mu = psP.tile([1, LC], F32, tag="gen", bufs=2)
                    nc.tensor.matmul(mu[:], mred1_sb[:], xs[:, cs],
                                     start=True, stop=True)
                    mu_sb = pA.tile([1, LC], F32, tag="musb")
                    nc.scalar.copy(mu_sb[:], mu[:])
                    mub = psP.tile([128, LC], F32, tag="gen", bufs=2)
                    nc.tensor.matmul(mub[:], onesr_sb[:], mu_sb[:],
                                     start=True, stop=True)
                    xc = pA.tile([128, LC], F32, tag="xc")
                    nc.vector.tensor_tensor(xc[:], xs[:, cs], mub[:], op=ALU.subtract)
                    xsq = pA.tile([128, LC], F32, tag="xsq")
                    nc.scalar.square(xsq[:], xc[:])
                    var = psP.tile([1, LC], F32, tag="gen", bufs=2)
                    nc.tensor.matmul(var[:], mred1_sb[:], xsq[:], start=True, stop=True)
                    sd = pA.tile([1, LC], F32, tag="sd")
                    nc.scalar.activation(sd[:], var[:], AF.Sqrt, bias=eps_sb[:, 0:1])
                    rstd = pA.tile([1, LC], F32, tag="rstd")
                    nc.vector.reciprocal(rstd[:], sd[:])
                    rstdb = psP.tile([128, LC], F32, tag="gen", bufs=2)
                    nc.tensor.matmul(rstdb[:], onesr_sb[:], rstd[:],
                                     start=True, stop=True)
                    xng = pA.tile([128, LC], F32, tag="xng")
                    nc.vector.tensor_tensor(xng[:], xc[:], rstdb[:], op=ALU.mult)
                    nc.scalar.activation(xnc[:, cs], xng[:], AF.Identity,
                                         bias=lnb_sb[:, 0:1], scale=lng_sb[:, 0:1])

                for c in range(NCH):
                    cs = slice(c * LC, (c + 1) * LC)
                    gps = psP.tile([128, LC], F32, tag="gen", bufs=2)
                    nc.tensor.matmul(gps[:], gateWT_sb[:], xnc[:, cs],
                                     start=True, stop=True)
                    nc.scalar.activation(gate[:, cs], gps[:], AF.Sigmoid,
                                         bias=gateb_sb[:, 0:1])
                    for gl in range(2):
                        rows = slice(gl * 64, gl * 64 + 64)
                        xzp = psP.tile([128, LC], F32, tag="gen", bufs=2)
                        nc.tensor.matmul(xzp[rows, :],
                                         winTu_sb[:, gl * DI:(gl + 1) * DI],
                                         xnc[:, cs], start=True, stop=True)
                        nc.scalar.copy(u_pad[rows, 3 + c * LC: 3 + (c + 1) * LC],
                                       xzp[rows, :])
                        xzp2 = psP.tile([128, LC], F32, tag="gen", bufs=2)
                        nc.tensor.matmul(xzp2[rows, :],
                                         winTz_sb[:, gl * DI:(gl + 1) * DI],
                                         xnc[:, cs], start=True, stop=True)
                        sgz = pA.tile([128, LC], BF, tag="sgz")
                        nc.scalar.activation(sgz[rows, :], xzp2[rows, :], AF.Sigmoid)
                        nc.vector.scalar_tensor_tensor(
                            zs[rows, cs], xzp2[rows, :], 0.0, sgz[rows, :],
                            op0=ALU.add, op1=ALU.mult)

            # ---- Phase B: per (group, direction, L-half) front-end + scan ----
            LH = L // 2
            NCC = LH // LC  # 4 front-end chunks per half
            with tc.tile_pool(name="pB", bufs=2) as wp:
                for gl in range(2):
                    rows = slice(gl * 64, gl * 64 + 64)
                    for dr in range(2):
                        i4 = gl * 2 + dr
                        h_prev = [None] * NJ
                        horder = (0, 1) if dr == 0 else (1, 0)
                        for hf in horder:
                            uc_h = wp.tile([128, LH], BF, tag="uc_h", bufs=2)
                            sgd_h = wp.tile([128, LH], BF, tag="sgd_h", bufs=2)
                            bc_h = wp.tile([DS, 2 * LH], BF, tag="bc_h", bufs=2)
                            # front-end (natural order); sigmoid table set
                            for cc in range(NCC):
                                c = hf * NCC + cc
                                ccs = slice(cc * LC, (cc + 1) * LC)
                                ucp = psP.tile([128, LC], F32, tag="gen", bufs=2)
                                for k in range(DC):
                                    off = (c * LC + k) if dr == 0 else (3 + c * LC + k)
                                    nc.tensor.matmul(
                                        ucp[:],
                                        conv4T_sb[rows,
                                                  (i4 * 4 + k) * 128:
                                                  (i4 * 4 + k + 1) * 128],
                                        u_pad[rows, off:off + LC],
                                        start=(k == 0), stop=(k == DC - 1))
                                sgu = wp.tile([128, LC], BF, tag="sgu")
                                nc.scalar.activation(sgu[:], ucp[:], AF.Sigmoid,
                                                     bias=convb_sb[:, i4:i4 + 1])
                                nc.vector.scalar_tensor_tensor(
                                    uc_h[:, ccs], ucp[:], convb_sb[:, i4:i4 + 1],
                                    sgu[:], op0=ALU.add, op1=ALU.mult)
                                dtp = psP.tile([128, LC], F32, tag="gen", bufs=2)
                                nc.tensor.matmul(dtp[:],
                                                 dtWT_sb[:, i4 * 128:(i4 + 1) * 128],
                                                 uc_h[0:DI, ccs],
                                                 start=True, stop=True)
                                nc.scalar.activation(sgd_h[:, ccs], dtp[:], AF.Sigmoid,
                                                     bias=dtb_sb[:, i4:i4 + 1],
                                                     scale=-1.0)
                                bcp = psP.tile([2 * DS, 2 * LC], F32, tag="bcp2",
                                               bufs=1)
                                nc.tensor.matmul(
                                    bcp[0:DS, 0:LC],
                                    xprojBCT_sb[:, i4 * 2 * DS:i4 * 2 * DS + DS],
                                    uc_h[0:DI, ccs], start=True, stop=True)
                                nc.tensor.matmul(
                                    bcp[0:DS, LC:2 * LC],
                                    xprojBCT_sb[:, i4 * 2 * DS + DS:(i4 + 1) * 2 * DS],
                                    uc_h[0:DI, ccs], start=True, stop=True)
                                bco = _AP(tensor=bc_h[:].tensor, offset=bc_h[:].offset
                                          + cc * LC,
                                          ap=[[2 * LH, DS], [LH, 2], [1, LC]])
                                nc.scalar.copy(bco, bcp[0:DS, :])
                            slot = (i4 * 2 + hf) % 4
                            nc.sync.dma_start(bc_dram[slot], bc_h[:])
                            # lnexp table set from here on
                            nc.scalar.activation(sgd_h[:], sgd_h[:], AF.Ln)
                            dt_h = sgd_h
                            dtuc = wp.tile([128, LH], BF, tag="dtuc", bufs=2)
                            nc.vector.tensor_tensor(dtuc[:], dt_h[:], uc_h[:],
                                                    op=ALU.mult)
                            ys = [psP.tile([128, LC], F32, tag=f"ys{q}", bufs=1,
                                           name=f"ys{q}")
                                  for q in range(NCC)]
                            for j in range(NJ):
                                dA = wp.tile([128, LH], BF, tag="dA")
                                nc.scalar.activation(
                                    dA[:], dt_h[:], AF.Exp,
                                    scale=asc_sb[:, i4 * NJ + j:i4 * NJ + j + 1])
                                bbs = wp.tile([128, 2 * LH], BF, tag="bbs")
                                bsrc = bc_dram[slot, 2 * j:2 * j + 2, :]
                                brep = _AP(tensor=bsrc.tensor, offset=bsrc.offset,
                                           ap=[[bsrc.ap[0][0], 2], [0, 64],
                                               [1, 2 * LH]])
                                dma_eng = (nc.sync, nc.gpsimd)[j % 2]
                                dma_eng.dma_start(bbs[:], brep)
                                dBu = wp.tile([128, LH], BF, tag="dBu")
                                nc.vector.tensor_tensor(dBu[:], dtuc[:],
                                                        bbs[:, 0:LH], op=ALU.mult)
                                h = hp.tile([128, LH], BF, tag="h")
                                first = (hf == horder[0])
                                hc = hp.tile([128, 1], BF, tag=f"hc{j}",
                                             name=f"hc{j}")
                                if dr == 0:
                                    init = 0.0 if first else h_prev[j][:, 0:1]
                                    nc.vector.tensor_tensor_scan(
                                        h[:], dA[:], dBu[:], init,
                                        op0=ALU.mult, op1=ALU.add)
                                    nc.scalar.copy(hc[:], h[:, LH - 1:LH])
                                else:
                                    init = 0.0 if first else h_prev[j][:, 0:1]
                                    nc.vector.tensor_tensor_scan(
                                        h[:, ::-1], dA[:, ::-1], dBu[:, ::-1], init,
                                        op0=ALU.mult, op1=ALU.add)
                                    nc.scalar.copy(hc[:], h[:, 0:1])
                                h_prev[j] = hc
                                prod = wp.tile([128, LH], BF, tag="prod")
                                nc.vector.tensor_tensor(prod[:], h[:],
                                                        bbs[:, LH:2 * LH],
                                                        op=ALU.mult)
                                for q in range(NCC):
                                    nc.tensor.matmul(
                                        ys[q][rows, :], mredM_sb[:, 0:DI],
                                        prod[:, q * LC:(q + 1) * LC],
                                        start=(j == 0), stop=(j == NJ - 1))
                            for q in range(NCC):
                                c = hf * NCC + q
                                cs = slice(c * LC, (c + 1) * LC)
                                ccs = slice(q * LC, (q + 1) * LC)
                                y1 = wp.tile([128, LC], BF, tag="y1")
                                nc.vector.scalar_tensor_tensor(
                                    y1[rows, :], uc_h[rows, ccs],
                                    dsk_sb[rows, i4:i4 + 1],
                                    ys[q][rows, :], op0=ALU.mult, op1=ALU.subtract)
                                if dr == 0:
                                    nc.vector.tensor_tensor(yfb[rows, cs],
                                                            y1[rows, :],
                                                            zs[rows, cs],
                                                            op=ALU.mult)
                                else:
                                    y2 = wp.tile([128, LC], BF, tag="y2")
                                    nc.vector.tensor_tensor(y2[rows, :], y1[rows, :],
                                                            zs[rows, cs],
                                                            op=ALU.mult)
                                    nc.vector.tensor_tensor(yfb[rows, cs],
                                                            yfb[rows, cs],
                                                            y2[rows, :], op=ALU.add)

            # ---- Phase C: Wout, exchange, blend, proj ----
            with tc.tile_pool(name="pC", bufs=2) as wpc:
                for c in range(NCH):
                    cs = slice(c * LC, (c + 1) * LC)
                    ymp = psP.tile([2 * DM, LC], F32, tag="gen", bufs=2)
                    nc.tensor.matmul(ymp[:], woutT_sb[:], yfb[:, cs],
                                     start=True, stop=True)
                    ym_sb = wpc.tile([2 * DM, LC], F32, tag="ymsb")
                    nc.scalar.copy(ym_sb[:], ymp[:])
                    nc.sync.dma_start(xm_loc[:, cs], ym_sb[:])
                nc.gpsimd.collective_compute(
                    "AllGather", ALU.bypass,
                    replica_groups=[[0, 1], [2, 3], [4, 5], [6, 7]],
                    ins=[xm_loc[:]], outs=[xm_all[:]])
                for c in range(NCH):
                    cs = slice(c * LC, (c + 1) * LC)
                    xm_t = wpc.tile([C, LC], F32, tag="xmt")
                    nc.sync.dma_start(xm_t[:], xm_all[:, cs])
                    ta = wpc.tile([128, LC], F32, tag="ta")
                    nc.vector.tensor_tensor(ta[:], xm_t[:], xs[:, cs],
                                            op=ALU.subtract)
                    tb2 = wpc.tile([128, LC], F32, tag="tb")
                    nc.vector.tensor_tensor(tb2[:], gate[:, cs], ta[:], op=ALU.mult)
                    tc2 = wpc.tile([128, LC], F32, tag="tc")
                    nc.vector.tensor_tensor(tc2[:], xs[:, cs], tb2[:], op=ALU.add)
                    op_ = psP.tile([128, LC], F32, tag="gen", bufs=2)
                    nc.tensor.matmul(op_[:], projT_sb[:], tc2[:], start=True, stop=True)
                    osb = wpc.tile([128, LC], F32, tag="osb")
                    nc.scalar.activation(osb[:], op_[:], AF.Identity,
                                         bias=projb_sb[:, 0:1])
                    nc.sync.dma_start(outp[:, cs], osb[:])
    nc.finalize()
    return nc


def _bf(a):
    import concourse.mybir as _mb
    return np.asarray(a).astype(_mb.dt.np(_mb.dt.bfloat16))


def _prep_inputs(inputs):
    """Build the 8 per-core in_maps from full inputs."""
    ii = {k: np.asarray(v, dtype=np.float32) for k, v in inputs.items()}
    x = ii["x"]

    maps_w = []  # weight dicts per group-set gs=0,1
    for gs in range(2):
        w = {}
        w9 = np.zeros((C, 9 * 128), np.float32)
        for tap in range(9):
            dy, dx = tap // 3, tap % 3
            blk = np.zeros((C, 128), np.float32)
            np.fill_diagonal(blk, ii["pos_conv_w"][:, 0, dy, dx])
            if tap == 4:
                blk[np.arange(C), np.arange(C)] += 1.0
            w9[:, tap * 128:(tap + 1) * 128] = blk
        w["w9"] = w9
        w["pe_b"] = np.ascontiguousarray(ii["pos_embed"][0].T) \
            + ii["pos_conv_b"][:, None]
        w["mred1"] = np.full((128, 1), 1.0 / 128, np.float32)
        w["onesr"] = np.ones((1, 128), np.float32)
        w["ln_g"] = np.ascontiguousarray(ii["ln_g"][:, None])
        w["ln_b"] = np.ascontiguousarray(ii["ln_b"][:, None])
        w["gateWT"] = np.ascontiguousarray(ii["gate_W"].T)
        w["gateb"] = np.ascontiguousarray(ii["gate_b"][:, None])
        w["projT"] = np.ascontiguousarray(ii["proj_W"].T)
        w["projb"] = np.ascontiguousarray(ii["proj_b"][:, None])
        w["mredM"] = _bf(np.tile(np.eye(DI, dtype=np.float32), (2, 1)))
        winTu = np.zeros((2, C, DI), np.float32)
        winTz = np.zeros((2, C, DI), np.float32)
        conv4T = np.zeros((2, 2, DC, DI, 128), np.float32)
        convb = np.zeros((2, 2, 128, 1), np.float32)
        dtWT = np.zeros((2, 2, DI, 128), np.float32)
        dtb = np.zeros((2, 2, 128, 1), np.float32)
        xprojBCT = np.zeros((2, 2, DI, 2 * DS), np.float32)
        A_sc = np.zeros((2, 2, 128, NJ), np.float32)
        dsk = np.zeros((2, 2, 128, 1), np.float32)
        woutT = np.zeros((128, 2 * DM), np.float32)
        for gl in range(2):
            gg = gs * 2 + gl
            gsl = slice(gg * DM, (gg + 1) * DM)
            winTu[gl, gsl, :] = ii["m_Win"][gg, 0:DI, :].T
            winTz[gl, gsl, :] = ii["m_Win"][gg, DI:2 * DI, :].T
            woutT[gl * 64:(gl + 1) * 64, gl * DM:(gl + 1) * DM] = ii["m_Wout"][gg].T
            for dr in range(2):
                for k in range(DC):
                    wk = ii["conv_w"][gg, dr, :, k if dr == 0 else DC - 1 - k]
                    blk = np.zeros((DI, 128), np.float32)
                    blk[np.arange(DI), np.arange(DI)] = wk
                    blk[np.arange(DI), 64 + np.arange(DI)] = wk
                    conv4T[gl, dr, k] = blk
                convb[gl, dr, :, 0] = np.tile(ii["conv_b"][gg, dr], 2)
                M2 = ii["dt_W"][gg, dr] @ ii["xproj_W"][gg, dr][0:DTR, :]  # (DI, DI)
                dtWT[gl, dr] = np.concatenate([M2.T, M2.T], axis=1)  # [DI, 128]
                dtb[gl, dr, :, 0] = -np.tile(ii["dt_b"][gg, dr], 2)
                xprojBCT[gl, dr] = ii["xproj_W"][gg, dr][DTR:DTR + 2 * DS, :].T
                A = np.exp(ii["A_log"][gg, dr])  # (DI, DS); dt is negated, so +exp
                p = np.arange(128)
                for j in range(NJ):
                    A_sc[gl, dr, :, j] = A[p % 64, 2 * j + p // 64]
                dsk[gl, dr, :, 0] = np.tile(ii["Dskip"][gg, dr], 2)
        w.update(winTu=winTu, winTz=winTz, conv4T=_bf(conv4T), convb=convb,
                 dtWT=_bf(dtWT), dtb=dtb, xprojBCT=_bf(xprojBCT), A_sc=A_sc,
                 dsk=dsk, woutT=_bf(woutT))
        maps_w.append(w)

    in_maps = []
    for k in range(NCORE):
        b, gs = k // 2, k % 2
        m = dict(maps_w[gs])
        xp = np.zeros((C, 66, 66), np.float32)
        xp[:, 1:65, 1:65] = x[b]
        m["xpad"] = np.ascontiguousarray(xp.reshape(C, 66 * 66))
        in_maps.append(m)
    return in_maps


_CACHE = {}


def kernel(**inputs):
    from concourse.bass_utils import run_bass_kernel_spmd
    if "nc" not in _CACHE:
        _CACHE["nc"] = _build_nc()
    nc = _CACHE["nc"]
    in_maps = _prep_inputs(inputs)
    res = run_bass_kernel_spmd(nc, in_maps, list(range(NCORE))).results
    out = np.stack([np.asarray(res[2 * b]["outp"]).reshape(OUT, H, W)
                    for b in range(B)])
    return out.astype(np.float32)
